# revision 51
# baseline (speedup 1.0000x reference)
"""Trainium2 Bass kernel for nn_ActionPredictionNet (GNN message passing).

Data-parallel over batch*particles: 8 NeuronCores, each handling 256
independent fully-connected 10-node particle graphs (2560 nodes, 23040
edges). The fully-connected structure lets us restructure the math:

  - Edge-MLP layer 1 collapses: e_in = [n[s], n[r]] so layer-1 pre-act is
    u[s] + v[r] with u = W_top^T n, v = W_bot^T n computed per NODE
    (2560 cols) instead of per EDGE (25600 cols), then a broadcast-add.
  - Edges are only consumed via the mean over incoming messages, so edge
    layer 3 folds into the aggregation: accumulate (sum_s h2_s) @ (w_e3/9)
    minus the diagonal term @ (w_e3/9) directly in PSUM.
  - All (s, r) pairs including s == r are computed (100 per graph instead of
    90 - perfectly regular layout), and the s == r diagonal is subtracted.

Layouts (per core, feat-major: features on SBUF partitions):
  - node tensors [128, 2560], column = a*256 + p  (a: node-in-graph 0..9,
    p: graph 0..255)  -> broadcast APs get innermost unit stride.
  - edge tensors [128, 25600], column = s*2560 + r*256 + p.

All matmuls fp16 x fp16 -> fp32 PSUM; biases folded on the host where
possible; activations stored fp16.
"""

import numpy as np

B, P, A = 32, 64, 10
S_DIM, H_DIM, MID = 64, 64, 128
ACT = 8
N_CORES = 8
NP_CORE = B * P // N_CORES          # 256 particle-graphs per core
NODES = NP_CORE * A                 # 2560 nodes per core
ECOLS = NP_CORE * A * A             # 25600 (s, r, p) edge columns per core

_PROG = None        # cached compiled program: (nc, meta)
LAST_EXEC_NS = None  # filled when KERNEL_TRACE=1


# ---------------------------------------------------------------- host utils

def _expected_edges():
    a = np.arange(A)
    s, r = np.meshgrid(a, a, indexing="ij")
    m = s != r
    s, r = s[m], r[m]
    offs = (np.arange(B * P) * A)[:, None]
    return (offs + s[None, :]).reshape(-1).astype(np.int64), \
           (offs + r[None, :]).reshape(-1).astype(np.int64)


def _to_ap_major(x_core):
    """[2560, D] in (p, a) node order -> [D, 2560] feat-major, (a, p) cols."""
    return np.ascontiguousarray(
        x_core.reshape(NP_CORE, A, -1).transpose(1, 0, 2).reshape(NODES, -1).T
    )


def _from_ap_major(out_core):
    """[ACT, 2560] feat-major (a, p) cols -> [2560, ACT] in (p, a) order."""
    return out_core.T.reshape(A, NP_CORE, ACT).transpose(1, 0, 2).reshape(NODES, ACT)


def _fallback_numpy(theta, s, i, senders, receivers,
                    w_in1, b_in1, w_in2, b_in2,
                    w_e1, b_e1, w_e2, b_e2, w_e3, b_e3,
                    w_n1, b_n1, w_n2, b_n2, w_n3, b_n3,
                    w_l1, b_l1, w_l2, b_l2):
    """fp32 numpy replica of the reference; used only if inputs deviate from
    the documented structure (non-fully-connected edges or non-constant i)."""
    N = B * P * A
    relu = lambda x: np.maximum(x, 0.0)
    x = np.concatenate([theta.reshape(N, H_DIM), s.reshape(N, S_DIM),
                        i.reshape(N, 1)], axis=-1).astype(np.float32)
    n = relu(x @ w_in1 + b_in1) @ w_in2 + b_in2
    e_in = np.concatenate([n[senders], n[receivers]], axis=-1)
    e = relu(e_in @ w_e1 + b_e1)
    e = relu(e @ w_e2 + b_e2)
    e = e @ w_e3 + b_e3
    agg = np.zeros((N, e.shape[1]), np.float32)
    np.add.at(agg, receivers, e)
    agg /= (A - 1)
    h = np.concatenate([n, agg], axis=-1)
    h = relu(h @ w_n1 + b_n1)
    h = relu(h @ w_n2 + b_n2)
    h = h @ w_n3 + b_n3
    out = relu(h @ w_l1 + b_l1) @ w_l2 + b_l2
    return out.reshape(B, P, A, ACT).astype(np.float32)


# ------------------------------------------------------------- device program

# weight-pack slot indices (all linear-linear layer pairs folded on host:
# w_in2 into we1t/we1b/wn1a, w_e3/9 into wn1b, w_n3 into w_l1)
W1T, WE1T, WE1B, WE2, WN1A, WN1B, WN1BN, WN2, WL1, WL2 = range(10)
NSLOTS = 10
# bias-pack column indices
B1, BU, BV, BE2, BN1, BN2, BL1, BL2 = range(8)


def _build_program():
    import concourse.bass as bass
    import concourse.mybir as mybir
    import concourse.tile as tile
    from concourse import bacc

    f16 = mybir.dt.float16
    f32 = mybir.dt.float32
    Af = mybir.ActivationFunctionType
    Op = mybir.AluOpType

    nc = bacc.Bacc("TRN2", target_bir_lowering=False, debug=False)
    x_dram = nc.dram_tensor("x_fm", [128, NODES], f16, kind="ExternalInput").ap()
    w_dram = nc.dram_tensor("w_pack", [128, NSLOTS * 128], f16,
                            kind="ExternalInput").ap()
    b_dram = nc.dram_tensor("b_pack", [128, 8], f32, kind="ExternalInput").ap()
    out_dram = nc.dram_tensor("out", [ACT, NODES], f32, kind="ExternalOutput").ap()

    with tile.TileContext(nc) as tc:
        with (
            tc.tile_pool(name="consts", bufs=1) as consts,
            tc.tile_pool(name="bigs", bufs=1) as bigs,
            tc.tile_pool(name="psA", bufs=3, space="PSUM") as psA,
            tc.tile_pool(name="psB", bufs=2, space="PSUM") as psB,
            tc.tile_pool(name="hsum", bufs=2) as hsum_pool,
        ):
            wt = consts.tile([128, NSLOTS * 128], f16, tag="wt")
            bt = consts.tile([128, 8], f32, tag="bt")
            x_fm = bigs.tile([128, NODES], f16, tag="x_fm")
            dummy = consts.tile([128, 512], f16, tag="dummy")
            dsink = consts.tile([128, 8], f32, tag="dsink")
            nc.sync.dma_start(out=wt[:], in_=w_dram)
            nc.sync.dma_start(out=bt[:], in_=b_dram)
            for x0 in range(0, NODES, 1024):
                x1 = min(x0 + 1024, NODES)
                nc.scalar.dma_start(out=x_fm[:, x0:x1], in_=x_dram[:, x0:x1])

            W = lambda k: wt[:, k * 128:(k + 1) * 128]
            bias = lambda k: bt[:, k:k + 1]

            # HAM warm-up: keep the PE array busy from t~0 so real matmuls
            # run at full clock. Dummy matmuls on a memset tile; one tiny
            # eviction keeps the chain live.
            nc.vector.memset(dummy[:], 0.0)
            # touch the Relu table set early so ACT_TABLE_LOAD hides in the
            # DMA-wait head instead of stalling the first real eviction
            nc.scalar.activation(dsink[:, 0:1], dummy[:, 0:2].bitcast(f32),
                                 Af.Relu)
            _first_dummy = [True]

            def pe_filler(n=2, src=None):
                # Dummy matmuls to keep the PE HAM clock warm. When `src` is
                # given, the filler reads freshly produced data so it only
                # becomes schedulable at that point of the pipeline (a filler
                # with no deps runs immediately, clustering at t=0).
                dfill = psB.tile([128, 512], f32, tag="psB")
                mv = dummy[:] if src is None else src
                for _ in range(n):
                    nc.tensor.matmul(dfill[:], dummy[:, :128], mv,
                                     start=True, stop=True)
                nc.vector.tensor_copy(dsink[:], dfill[:, :8])

            pe_filler(10)

            t_enc = bigs.tile([128, NODES], f16, tag="t_enc")
            u_t = bigs.tile([128, NODES], f16, tag="u_t")
            v_t = bigs.tile([128, NODES], f16, tag="v_t")
            h1_t = bigs.tile([128, ECOLS], f16, tag="h1_t")
            h2_t = bigs.tile([128, ECOLS], f16, tag="h2_t")
            t_n1 = bigs.tile([128, NODES], f16, tag="t_n1")
            t_n2 = bigs.tile([128, NODES], f16, tag="t_n2")
            t_l1 = bigs.tile([128, NODES], f16, tag="t_l1")
            out_sb = bigs.tile([ACT, NODES], f32, tag="out_sb")

            def evict(eng, dst, src, bias_ap, relu):
                if eng == "act":
                    if relu:
                        nc.scalar.activation(dst, src, Af.Relu, bias=bias_ap)
                    elif bias_ap is not None:
                        nc.scalar.activation(dst, src, Af.Identity, bias=bias_ap)
                    else:
                        nc.scalar.copy(dst, src)
                else:
                    if relu:
                        nc.vector.tensor_scalar(dst, src, bias_ap, 0.0,
                                                Op.add, Op.max)
                    elif bias_ap is not None:
                        nc.vector.tensor_scalar_add(dst, src, bias_ap)
                    else:
                        nc.vector.tensor_copy(dst, src)

            def group(srcs, dst, g0, gw, bias_idx, relu, eng):
                """One 1024-col PSUM group of a dense layer: accumulate
                matmuls from (weight-slot, src-tile) pairs, then evict."""
                ps = psA.tile([128, 1024], f32, tag="psA")
                for o in range(0, gw, 512):
                    nw = min(512, gw - o)
                    for si, (wk, src) in enumerate(srcs):
                        nc.tensor.matmul(
                            ps[:, o:o + nw], W(wk),
                            src[:, g0 + o:g0 + o + nw],
                            start=(si == 0), stop=(si == len(srcs) - 1))
                evict(eng, dst[:, g0:g0 + gw], ps[:, :gw],
                      bias(bias_idx) if bias_idx is not None else None, relu)

            def layer(srcs, dst, bias_idx, relu, engines):
                for gi, g0 in enumerate(range(0, NODES, 1024)):
                    gw = min(1024, NODES - g0)
                    group(srcs, dst, g0, gw, bias_idx, relu,
                          engines[gi % len(engines)])

            # ---- node encoder + edge layer-1 node halves (w_in2 folded in)
            layer([(W1T, x_fm)], t_enc, B1, True, ["act"])
            pe_filler()
            layer([(WE1B, t_enc)], v_t, BV, False, ["vec"])
            pe_filler()
            layer([(WE1T, t_enc)], u_t, BU, False, ["act"])
            pe_filler()

            # ---- h1 = relu(u[s] + v[r]) over (r, s, p) columns.
            # First two pairs at single-r granularity so the h2 stream can
            # start as soon as possible.
            u4 = u_t[:].rearrange("f (s p) -> f s p", p=NP_CORE).unsqueeze(1)
            v3 = v_t[:].rearrange("f (r p) -> f r p", p=NP_CORE)

            def tt_piece(r0, rc, relu_eng):
                ub = u4.broadcast_to([128, rc, A, NP_CORE])
                vb = v3[:, r0:r0 + rc, :].unsqueeze(2) \
                    .broadcast_to([128, rc, A, NP_CORE])
                w0 = r0 * A * NP_CORE
                w1 = (r0 + rc) * A * NP_CORE
                o4 = h1_t[:, w0:w1].rearrange("f (r s p) -> f r s p",
                                              s=A, p=NP_CORE)
                nc.vector.tensor_add(o4, vb, ub)
                flat = h1_t[:, w0:w1]
                if relu_eng == "act":
                    nc.scalar.activation(flat, flat, Af.Relu)
                else:
                    nc.vector.tensor_scalar_max(flat, flat, 0.0)

            for r0 in range(A):
                tt_piece(r0, 1, "vec")
                pe_filler(1 if r0 % 2 == 0 else 2)

            # ---- main stream: h2 = relu(w_e2^T h1 + b_e2); per r-block pair
            # the fused agg+n1 chunk:
            #   t_n1 = relu(wn1a'^T t_enc + wn1b'^T (sum_s h2_s - diag) + b_n1)
            # and per completed t_n1 slab, the rest of the network.
            h2r = h2_t[:].rearrange("f (r q) -> f r q", q=A * NP_CORE)
            hs_mov = lambda c, s_: h2r[:, 2 * c:2 * c + 2,
                                       s_ * NP_CORE:(s_ + 1) * NP_CORE]

            def slab(k, s0, sw, step=512):
                """node-MLP tail + decoder for t_n1 cols [s0, s0+sw),
                pipelined in `step`-col stages."""
                for c0 in range(s0, s0 + sw, step):
                    group([(WN2, t_n1)], t_n2, c0, step, BN2, True, "vec")
                    group([(WL1, t_n2)], t_l1, c0, step, BL1, True, "act")
                    ps = psB.tile([128, 512], f32, tag="psB")
                    nc.tensor.matmul(ps[:, :step], W(WL2), t_l1[:, c0:c0 + step],
                                     start=True, stop=True)
                    nc.scalar.activation(out_sb[:, c0:c0 + step],
                                         ps[:ACT, :step], Af.Identity,
                                         bias=bt[0:ACT, BL2:BL2 + 1])
                nc.sync.dma_start(out=out_dram[:, s0:s0 + sw],
                                  in_=out_sb[:, s0:s0 + sw])

            for g in range(ECOLS // 1024):
                g0 = g * 1024
                ps = psA.tile([128, 1024], f32, tag="psA")
                for o in range(0, 1024, 512):
                    nc.tensor.matmul(ps[:, o:o + 512], W(WE2),
                                     h1_t[:, g0 + o:g0 + o + 512],
                                     start=True, stop=True)
                evict("vec" if g % 3 == 2 else "act",
                      h2_t[:, g0:g0 + 1024], ps[:], bias(BE2), True)
                if g < 2:
                    pe_filler(1)
                if g % 5 == 4:
                    c = g // 5          # agg+n1 chunk: r in {2c, 2c+1}
                    psb = psB.tile([128, 512], f32, tag="psB")
                    nc.tensor.matmul(psb[:], W(WN1A),
                                     t_enc[:, c * 512:(c + 1) * 512],
                                     start=True, stop=False)
                    if c < 4:
                        # sum_s h2_s via accumulating SWDGE DMAs (two
                        # parallel even/odd chains), then one matmul each.
                        hv = hsum_pool.tile([128, 2, 512], f16, tag="hsum")
                        for par in range(2):
                            dst = hv[:, par, :].rearrange(
                                "f (r p) -> f r p", p=NP_CORE)
                            for k in range(5):
                                s_ = 2 * k + par
                                nc.gpsimd.dma_start(
                                    out=dst, in_=hs_mov(c, s_),
                                    accum_op=(Op.bypass if k == 0 else Op.add))
                            nc.tensor.matmul(psb[:], W(WN1B), hv[:, par, :],
                                             start=False, stop=False)
                    else:
                        for s_ in range(A):
                            nc.tensor.matmul(psb[:], W(WN1B), hs_mov(c, s_),
                                             start=False, stop=False)
                    for j in range(2):      # diagonal s == r, r = 2c + j
                        r = 2 * c + j
                        d0 = r * (A * NP_CORE + NP_CORE)
                        nc.tensor.matmul(psb[:, j * 256:(j + 1) * 256],
                                         W(WN1BN),
                                         h2_t[:, d0:d0 + NP_CORE],
                                         start=False, stop=(j == 1))
                    evict("act", t_n1[:, c * 512:(c + 1) * 512], psb[:],
                          bias(BN1), True)
                    if c == 1:
                        slab(0, 0, 1024)
                    elif c == 3:
                        slab(1, 1024, 1024)
                    elif c == 4:
                        slab(2, 2048, 512, step=256)

    nc.compile()
    _dedupe_ldweights(nc)
    return nc


def _dedupe_ldweights(nc):
    """Remove redundant PE weight loads after bacc splits matmuls into
    Ldweights+Matmult pairs: a Ldweights whose source AP equals the
    previously loaded one (PE stream order == block order) is a no-op.
    Only drop instructions carrying no semaphore waits/updates."""
    from concourse import mybir
    for f in nc.m.functions:
        for b in f.blocks:
            last = None
            keep = []
            for i in b.instructions:
                if isinstance(i, mybir.InstLdweights):
                    key = str(i.ins[0])
                    if key == last and i.sync_info is None:
                        continue
                    last = key
                keep.append(i)
            if len(keep) != len(b.instructions):
                b.instructions[:] = keep


def _get_program():
    global _PROG
    if _PROG is None:
        _PROG = _build_program()
    return _PROG


# ------------------------------------------------------------------- kernel

def kernel(theta, s, i, senders, receivers,
           w_in1, b_in1, w_in2, b_in2,
           w_e1, b_e1, w_e2, b_e2, w_e3, b_e3,
           w_n1, b_n1, w_n2, b_n2, w_n3, b_n3,
           w_l1, b_l1, w_l2, b_l2):
    global LAST_EXEC_NS
    import os

    args = dict(theta=theta, s=s, i=i, senders=senders, receivers=receivers,
                w_in1=w_in1, b_in1=b_in1, w_in2=w_in2, b_in2=b_in2,
                w_e1=w_e1, b_e1=b_e1, w_e2=w_e2, b_e2=b_e2,
                w_e3=w_e3, b_e3=b_e3, w_n1=w_n1, b_n1=b_n1,
                w_n2=w_n2, b_n2=b_n2, w_n3=w_n3, b_n3=b_n3,
                w_l1=w_l1, b_l1=b_l1, w_l2=w_l2, b_l2=b_l2)
    args = {k: np.asarray(v) for k, v in args.items()}

    # The device program hardcodes the documented block-diagonal
    # fully-connected edge structure and constant-i input; verify, else
    # fall back to a host fp32 computation (correct for any input).
    exp_s, exp_r = _expected_edges()
    i_flat = np.asarray(args["i"], np.float32).reshape(-1)
    structured = (np.array_equal(np.asarray(args["senders"], np.int64), exp_s)
                  and np.array_equal(np.asarray(args["receivers"], np.int64), exp_r)
                  and np.all(i_flat == i_flat[0]))
    if not structured:
        return _fallback_numpy(**{k: np.asarray(v, np.float32)
                                  if np.asarray(v).dtype != np.int32 else np.asarray(v)
                                  for k, v in args.items()})

    f64 = np.float64
    w_in1_, b_in1_ = args["w_in1"].astype(f64), args["b_in1"].astype(f64)
    w_in2_, b_in2_ = args["w_in2"].astype(f64), args["b_in2"].astype(f64)
    w_e1_, b_e1_ = args["w_e1"].astype(f64), args["b_e1"].astype(f64)
    w_e3_, b_e3_ = args["w_e3"].astype(f64), args["b_e3"].astype(f64)
    w_n1_, b_n1_ = args["w_n1"].astype(f64), args["b_n1"].astype(f64)
    w_n3_, b_n3_ = args["w_n3"].astype(f64), args["b_n3"].astype(f64)
    w_l1_, b_l1_ = args["w_l1"].astype(f64), args["b_l1"].astype(f64)

    b1_eff = b_in1_ + i_flat[0] * w_in1_[H_DIM + S_DIM]
    b_u = b_e1_ + b_in2_ @ w_e1_[:MID]
    b_v = b_in2_ @ w_e1_[MID:]
    b_n1_eff = b_n1_ + b_in2_ @ w_n1_[:MID] + b_e3_ @ w_n1_[MID:]
    b_l1_eff = b_l1_ + b_n3_ @ w_l1_

    wn1b_f = (w_e3_ / (A - 1)) @ w_n1_[MID:]
    wslots = np.zeros((NSLOTS, 128, 128), np.float16)
    wslots[W1T] = w_in1_[:128].astype(np.float16)
    wslots[WE1T] = (w_in2_ @ w_e1_[:MID]).astype(np.float16)
    wslots[WE1B] = (w_in2_ @ w_e1_[MID:]).astype(np.float16)
    wslots[WE2] = args["w_e2"].astype(np.float16)
    wslots[WN1A] = (w_in2_ @ w_n1_[:MID]).astype(np.float16)
    wslots[WN1B] = wn1b_f.astype(np.float16)
    wslots[WN1BN] = (-wn1b_f).astype(np.float16)
    wslots[WN2] = args["w_n2"].astype(np.float16)
    wslots[WL1] = (w_n3_ @ w_l1_).astype(np.float16)
    wslots[WL2, :, :ACT] = args["w_l2"].astype(np.float16)
    w_pack = np.ascontiguousarray(
        wslots.transpose(1, 0, 2).reshape(128, NSLOTS * 128))

    b_pack = np.zeros((128, 8), np.float32)
    for idx, vec in ((B1, b1_eff), (BU, b_u), (BV, b_v),
                     (BE2, args["b_e2"]), (BN1, b_n1_eff),
                     (BN2, args["b_n2"]), (BL1, b_l1_eff)):
        b_pack[:, idx] = np.asarray(vec, np.float32)
    b_pack[:ACT, BL2] = args["b_l2"].astype(np.float32)

    # node features, feat-major, (a, p) column order, per-core shards
    n_all = B * P * A
    X = np.concatenate([args["theta"].reshape(n_all, H_DIM),
                        args["s"].reshape(n_all, S_DIM)], axis=-1)
    in_maps = []
    for c in range(N_CORES):
        xc = X[c * NODES:(c + 1) * NODES]
        in_maps.append({
            "x_fm": _to_ap_major(xc).astype(np.float16),
            "w_pack": w_pack,
            "b_pack": b_pack,
        })

    nc = _get_program()
    if os.environ.get("KERNEL_SIM", "0") == "1":
        # CoreSim core 0 only (cores are identical up to data); other cores
        # return zeros. For correctness devloop, not grading.
        from concourse import bass_interp
        sim = bass_interp.CoreSim(nc)
        for k, v in in_maps[0].items():
            sim.tensor(k)[:] = v
        sim.simulate()
        results = [{"out": np.array(sim.tensor("out"))}]
        results += [{"out": np.zeros((ACT, NODES), np.float32)}
                    for _ in range(N_CORES - 1)]
        parts = [_from_ap_major(r["out"]) for r in results]
        return np.concatenate(parts, axis=0).reshape(B, P, A, ACT).astype(np.float32)

    from concourse.bass_utils import run_bass_kernel_spmd
    trace = os.environ.get("KERNEL_TRACE", "0") == "1"
    res = run_bass_kernel_spmd(nc, in_maps, core_ids=list(range(N_CORES)),
                               trace=trace)
    LAST_EXEC_NS = res.exec_time_ns

    parts = [_from_ap_major(res.results[c]["out"]) for c in range(N_CORES)]
    return np.concatenate(parts, axis=0).reshape(B, P, A, ACT).astype(np.float32)


# revision 55
# speedup vs baseline: 1.9042x; 1.9042x over previous
"""Trainium2 Bass kernel for nn_ActionPredictionNet (GNN message passing).

Data-parallel over batch*particles: 8 NeuronCores, each handling 256
independent fully-connected 10-node particle graphs (2560 nodes, 23040
edges). The fully-connected structure lets us restructure the math:

  - Edge-MLP layer 1 collapses: e_in = [n[s], n[r]] so layer-1 pre-act is
    u[s] + v[r] with u = W_top^T n, v = W_bot^T n computed per NODE
    (2560 cols) instead of per EDGE (25600 cols), then a broadcast-add.
  - Edges are only consumed via the mean over incoming messages, so edge
    layer 3 folds into the aggregation: accumulate (sum_s h2_s) @ (w_e3/9)
    minus the diagonal term @ (w_e3/9) directly in PSUM.
  - All (s, r) pairs including s == r are computed (100 per graph instead of
    90 - perfectly regular layout), and the s == r diagonal is subtracted.

Layouts (per core, feat-major: features on SBUF partitions):
  - node tensors [128, 2560], column = a*256 + p  (a: node-in-graph 0..9,
    p: graph 0..255)  -> broadcast APs get innermost unit stride.
  - edge tensors [128, 25600], column = s*2560 + r*256 + p.

All matmuls fp16 x fp16 -> fp32 PSUM; biases folded on the host where
possible; activations stored fp16.
"""

import numpy as np

B, P, A = 32, 64, 10
S_DIM, H_DIM, MID = 64, 64, 128
ACT = 8
N_CORES = 8
NP_CORE = B * P // N_CORES          # 256 particle-graphs per core
NODES = NP_CORE * A                 # 2560 nodes per core
ECOLS = NP_CORE * A * A             # 25600 (s, r, p) edge columns per core

_PROG = None        # cached compiled program: (nc, meta)
LAST_EXEC_NS = None  # filled when KERNEL_TRACE=1


# ---------------------------------------------------------------- host utils

def _expected_edges():
    a = np.arange(A)
    s, r = np.meshgrid(a, a, indexing="ij")
    m = s != r
    s, r = s[m], r[m]
    offs = (np.arange(B * P) * A)[:, None]
    return (offs + s[None, :]).reshape(-1).astype(np.int64), \
           (offs + r[None, :]).reshape(-1).astype(np.int64)


def _to_ap_major(x_core):
    """[2560, D] in (p, a) node order -> [D, 2560] feat-major, (a, p) cols."""
    return np.ascontiguousarray(
        x_core.reshape(NP_CORE, A, -1).transpose(1, 0, 2).reshape(NODES, -1).T
    )


def _from_ap_major(out_core):
    """[ACT, 2560] feat-major (a, p) cols -> [2560, ACT] in (p, a) order."""
    return out_core.T.reshape(A, NP_CORE, ACT).transpose(1, 0, 2).reshape(NODES, ACT)


def _fallback_numpy(theta, s, i, senders, receivers,
                    w_in1, b_in1, w_in2, b_in2,
                    w_e1, b_e1, w_e2, b_e2, w_e3, b_e3,
                    w_n1, b_n1, w_n2, b_n2, w_n3, b_n3,
                    w_l1, b_l1, w_l2, b_l2):
    """fp32 numpy replica of the reference; used only if inputs deviate from
    the documented structure (non-fully-connected edges or non-constant i)."""
    N = B * P * A
    relu = lambda x: np.maximum(x, 0.0)
    x = np.concatenate([theta.reshape(N, H_DIM), s.reshape(N, S_DIM),
                        i.reshape(N, 1)], axis=-1).astype(np.float32)
    n = relu(x @ w_in1 + b_in1) @ w_in2 + b_in2
    e_in = np.concatenate([n[senders], n[receivers]], axis=-1)
    e = relu(e_in @ w_e1 + b_e1)
    e = relu(e @ w_e2 + b_e2)
    e = e @ w_e3 + b_e3
    agg = np.zeros((N, e.shape[1]), np.float32)
    np.add.at(agg, receivers, e)
    agg /= (A - 1)
    h = np.concatenate([n, agg], axis=-1)
    h = relu(h @ w_n1 + b_n1)
    h = relu(h @ w_n2 + b_n2)
    h = h @ w_n3 + b_n3
    out = relu(h @ w_l1 + b_l1) @ w_l2 + b_l2
    return out.reshape(B, P, A, ACT).astype(np.float32)


# ------------------------------------------------------------- device program

# weight-pack slot indices (all linear-linear layer pairs folded on host:
# w_in2 into we1t/we1b/wn1a, w_e3/9 into wn1b, w_n3 into w_l1)
W1T, WE1T, WE1B, WE2, WN1A, WN1B, WN1BN, WN2, WL1, WL2 = range(10)
NSLOTS = 10
# bias-pack column indices
B1, BU, BV, BE2, BN1, BN2, BL1, BL2 = range(8)


def _build_program():
    import concourse.bass as bass
    import concourse.mybir as mybir
    import concourse.tile as tile
    from concourse import bacc

    f16 = mybir.dt.float16
    f32 = mybir.dt.float32
    Af = mybir.ActivationFunctionType
    Op = mybir.AluOpType

    nc = bacc.Bacc("TRN2", target_bir_lowering=False, debug=False)
    x_dram = nc.dram_tensor("x_fm", [128, NODES], f16, kind="ExternalInput").ap()
    w_dram = nc.dram_tensor("w_pack", [128, NSLOTS * 128], f16,
                            kind="ExternalInput").ap()
    b_dram = nc.dram_tensor("b_pack", [128, 8], f32, kind="ExternalInput").ap()
    out_dram = nc.dram_tensor("out", [ACT, NODES], f32, kind="ExternalOutput").ap()

    with tile.TileContext(nc) as tc:
        with (
            tc.tile_pool(name="consts", bufs=1) as consts,
            tc.tile_pool(name="bigs", bufs=1) as bigs,
            tc.tile_pool(name="psA", bufs=3, space="PSUM") as psA,
            tc.tile_pool(name="psB", bufs=2, space="PSUM") as psB,
        ):
            wt = consts.tile([128, NSLOTS * 128], f16, tag="wt")
            bt = consts.tile([128, 8], f32, tag="bt")
            x_fm = bigs.tile([128, NODES], f16, tag="x_fm")
            dummy = consts.tile([128, 512], f16, tag="dummy")
            dsink = consts.tile([128, 8], f32, tag="dsink")
            nc.sync.dma_start(out=wt[:], in_=w_dram)
            nc.sync.dma_start(out=bt[:], in_=b_dram)
            for x0 in range(0, NODES, 1024):
                x1 = min(x0 + 1024, NODES)
                nc.scalar.dma_start(out=x_fm[:, x0:x1], in_=x_dram[:, x0:x1])

            W = lambda k: wt[:, k * 128:(k + 1) * 128]
            bias = lambda k: bt[:, k:k + 1]

            # HAM warm-up: keep the PE array busy from t~0 so real matmuls
            # run at full clock. Dummy matmuls on a memset tile; one tiny
            # eviction keeps the chain live.
            nc.vector.memset(dummy[:], 0.0)
            # touch the Relu table set early so ACT_TABLE_LOAD hides in the
            # DMA-wait head instead of stalling the first real eviction
            nc.scalar.activation(dsink[:, 0:1], dummy[:, 0:2].bitcast(f32),
                                 Af.Relu)
            _first_dummy = [True]

            def pe_filler(n=2, src=None):
                # Dummy matmuls to keep the PE HAM clock warm. When `src` is
                # given, the filler reads freshly produced data so it only
                # becomes schedulable at that point of the pipeline (a filler
                # with no deps runs immediately, clustering at t=0).
                dfill = psB.tile([128, 512], f32, tag="psB")
                mv = dummy[:] if src is None else src
                for _ in range(n):
                    nc.tensor.matmul(dfill[:], dummy[:, :128], mv,
                                     start=True, stop=True)
                nc.vector.tensor_copy(dsink[:], dfill[:, :8])

            pe_filler(10)

            t_enc = bigs.tile([128, NODES], f16, tag="t_enc")
            u_t = bigs.tile([128, NODES], f16, tag="u_t")
            v_t = bigs.tile([128, NODES], f16, tag="v_t")
            h1_t = bigs.tile([128, ECOLS], f16, tag="h1_t")
            h2_t = bigs.tile([128, ECOLS], f16, tag="h2_t")
            t_n1 = bigs.tile([128, NODES], f16, tag="t_n1")
            t_n2 = bigs.tile([128, NODES], f16, tag="t_n2")
            t_l1 = bigs.tile([128, NODES], f16, tag="t_l1")
            out_sb = bigs.tile([ACT, NODES], f32, tag="out_sb")

            def evict(eng, dst, src, bias_ap, relu):
                if eng == "act":
                    if relu:
                        nc.scalar.activation(dst, src, Af.Relu, bias=bias_ap)
                    elif bias_ap is not None:
                        nc.scalar.activation(dst, src, Af.Identity, bias=bias_ap)
                    else:
                        nc.scalar.copy(dst, src)
                else:
                    if relu:
                        nc.vector.tensor_scalar(dst, src, bias_ap, 0.0,
                                                Op.add, Op.max)
                    elif bias_ap is not None:
                        nc.vector.tensor_scalar_add(dst, src, bias_ap)
                    else:
                        nc.vector.tensor_copy(dst, src)

            def group(srcs, dst, g0, gw, bias_idx, relu, eng):
                """One 1024-col PSUM group of a dense layer: accumulate
                matmuls from (weight-slot, src-tile) pairs, then evict."""
                ps = psA.tile([128, 1024], f32, tag="psA")
                for o in range(0, gw, 512):
                    nw = min(512, gw - o)
                    for si, (wk, src) in enumerate(srcs):
                        nc.tensor.matmul(
                            ps[:, o:o + nw], W(wk),
                            src[:, g0 + o:g0 + o + nw],
                            start=(si == 0), stop=(si == len(srcs) - 1))
                evict(eng, dst[:, g0:g0 + gw], ps[:, :gw],
                      bias(bias_idx) if bias_idx is not None else None, relu)

            def layer(srcs, dst, bias_idx, relu, engines):
                for gi, g0 in enumerate(range(0, NODES, 1024)):
                    gw = min(1024, NODES - g0)
                    group(srcs, dst, g0, gw, bias_idx, relu,
                          engines[gi % len(engines)])

            # ---- node encoder + edge layer-1 node halves (w_in2 folded in)
            layer([(W1T, x_fm)], t_enc, B1, True, ["act"])
            pe_filler()
            layer([(WE1B, t_enc)], v_t, BV, False, ["vec"])
            pe_filler()
            layer([(WE1T, t_enc)], u_t, BU, False, ["act"])
            pe_filler()

            # ---- h1 = relu(u[s] + v[r]) over (r, s, p) columns.
            # First two pairs at single-r granularity so the h2 stream can
            # start as soon as possible.
            u4 = u_t[:].rearrange("f (s p) -> f s p", p=NP_CORE).unsqueeze(1)
            v3 = v_t[:].rearrange("f (r p) -> f r p", p=NP_CORE)

            def tt_piece(r0, rc, relu_eng):
                ub = u4.broadcast_to([128, rc, A, NP_CORE])
                vb = v3[:, r0:r0 + rc, :].unsqueeze(2) \
                    .broadcast_to([128, rc, A, NP_CORE])
                w0 = r0 * A * NP_CORE
                w1 = (r0 + rc) * A * NP_CORE
                o4 = h1_t[:, w0:w1].rearrange("f (r s p) -> f r s p",
                                              s=A, p=NP_CORE)
                nc.vector.tensor_add(o4, vb, ub)
                flat = h1_t[:, w0:w1]
                if relu_eng == "act":
                    nc.scalar.activation(flat, flat, Af.Relu)
                else:
                    nc.vector.tensor_scalar_max(flat, flat, 0.0)

            for r0 in range(4):
                tt_piece(r0, 1, "act" if r0 % 2 else "vec")
                pe_filler(1)
            for r0 in range(4, A, 2):
                tt_piece(r0, 2, "act" if r0 == 4 else "vec")
                pe_filler(3)

            # ---- main stream: h2 = relu(w_e2^T h1 + b_e2); per r-block pair
            # the fused agg+n1 chunk:
            #   t_n1 = relu(wn1a'^T t_enc + wn1b'^T (sum_s h2_s - diag) + b_n1)
            # and per completed t_n1 slab, the rest of the network.
            h2r = h2_t[:].rearrange("f (r q) -> f r q", q=A * NP_CORE)
            hs_mov = lambda c, s_: h2r[:, 2 * c:2 * c + 2,
                                       s_ * NP_CORE:(s_ + 1) * NP_CORE]

            def slab(k, s0, sw, step=512):
                """node-MLP tail + decoder for t_n1 cols [s0, s0+sw),
                pipelined in `step`-col stages."""
                for c0 in range(s0, s0 + sw, step):
                    group([(WN2, t_n1)], t_n2, c0, step, BN2, True, "vec")
                    group([(WL1, t_n2)], t_l1, c0, step, BL1, True, "act")
                    ps = psB.tile([128, 512], f32, tag="psB")
                    nc.tensor.matmul(ps[:, :step], W(WL2), t_l1[:, c0:c0 + step],
                                     start=True, stop=True)
                    nc.scalar.activation(out_sb[:, c0:c0 + step],
                                         ps[:ACT, :step], Af.Identity,
                                         bias=bt[0:ACT, BL2:BL2 + 1])
                nc.sync.dma_start(out=out_dram[:, s0:s0 + sw],
                                  in_=out_sb[:, s0:s0 + sw])

            for g in range(ECOLS // 1024):
                g0 = g * 1024
                ps = psA.tile([128, 1024], f32, tag="psA")
                for o in range(0, 1024, 512):
                    nc.tensor.matmul(ps[:, o:o + 512], W(WE2),
                                     h1_t[:, g0 + o:g0 + o + 512],
                                     start=True, stop=True)
                evict("vec" if g % 2 == 1 else "act",
                      h2_t[:, g0:g0 + 1024], ps[:], bias(BE2), True)
                if g < 2:
                    pe_filler(1)
                if g % 5 == 4:
                    c = g // 5          # agg+n1 chunk: r in {2c, 2c+1}
                    psb = psB.tile([128, 512], f32, tag="psB")
                    nc.tensor.matmul(psb[:], W(WN1A),
                                     t_enc[:, c * 512:(c + 1) * 512],
                                     start=True, stop=False)
                    for s_ in range(A):
                        nc.tensor.matmul(psb[:], W(WN1B), hs_mov(c, s_),
                                         start=False, stop=False)
                    for j in range(2):      # diagonal s == r, r = 2c + j
                        r = 2 * c + j
                        d0 = r * (A * NP_CORE + NP_CORE)
                        nc.tensor.matmul(psb[:, j * 256:(j + 1) * 256],
                                         W(WN1BN),
                                         h2_t[:, d0:d0 + NP_CORE],
                                         start=False, stop=(j == 1))
                    evict("act", t_n1[:, c * 512:(c + 1) * 512], psb[:],
                          bias(BN1), True)
                    if c == 1:
                        slab(0, 0, 1024)
                    elif c == 3:
                        slab(1, 1024, 1024)
                    elif c == 4:
                        slab(2, 2048, 512, step=256)

    nc.compile()
    _dedupe_ldweights(nc)
    return nc


def _dedupe_ldweights(nc):
    """Remove redundant PE weight loads after bacc splits matmuls into
    Ldweights+Matmult pairs: a Ldweights whose source AP equals the
    previously loaded one (PE stream order == block order) is a no-op.
    Only drop instructions carrying no semaphore waits/updates."""
    from concourse import mybir
    for f in nc.m.functions:
        for b in f.blocks:
            last = None
            keep = []
            for i in b.instructions:
                if isinstance(i, mybir.InstLdweights):
                    key = str(i.ins[0])
                    if key == last and i.sync_info is None:
                        continue
                    last = key
                keep.append(i)
            if len(keep) != len(b.instructions):
                b.instructions[:] = keep


def _get_program():
    global _PROG
    if _PROG is None:
        _PROG = _build_program()
    return _PROG


# ------------------------------------------------------------------- kernel

def kernel(theta, s, i, senders, receivers,
           w_in1, b_in1, w_in2, b_in2,
           w_e1, b_e1, w_e2, b_e2, w_e3, b_e3,
           w_n1, b_n1, w_n2, b_n2, w_n3, b_n3,
           w_l1, b_l1, w_l2, b_l2):
    global LAST_EXEC_NS
    import os

    args = dict(theta=theta, s=s, i=i, senders=senders, receivers=receivers,
                w_in1=w_in1, b_in1=b_in1, w_in2=w_in2, b_in2=b_in2,
                w_e1=w_e1, b_e1=b_e1, w_e2=w_e2, b_e2=b_e2,
                w_e3=w_e3, b_e3=b_e3, w_n1=w_n1, b_n1=b_n1,
                w_n2=w_n2, b_n2=b_n2, w_n3=w_n3, b_n3=b_n3,
                w_l1=w_l1, b_l1=b_l1, w_l2=w_l2, b_l2=b_l2)
    args = {k: np.asarray(v) for k, v in args.items()}

    # The device program hardcodes the documented block-diagonal
    # fully-connected edge structure and constant-i input; verify, else
    # fall back to a host fp32 computation (correct for any input).
    exp_s, exp_r = _expected_edges()
    i_flat = np.asarray(args["i"], np.float32).reshape(-1)
    structured = (np.array_equal(np.asarray(args["senders"], np.int64), exp_s)
                  and np.array_equal(np.asarray(args["receivers"], np.int64), exp_r)
                  and np.all(i_flat == i_flat[0]))
    if not structured:
        return _fallback_numpy(**{k: np.asarray(v, np.float32)
                                  if np.asarray(v).dtype != np.int32 else np.asarray(v)
                                  for k, v in args.items()})

    f64 = np.float64
    w_in1_, b_in1_ = args["w_in1"].astype(f64), args["b_in1"].astype(f64)
    w_in2_, b_in2_ = args["w_in2"].astype(f64), args["b_in2"].astype(f64)
    w_e1_, b_e1_ = args["w_e1"].astype(f64), args["b_e1"].astype(f64)
    w_e3_, b_e3_ = args["w_e3"].astype(f64), args["b_e3"].astype(f64)
    w_n1_, b_n1_ = args["w_n1"].astype(f64), args["b_n1"].astype(f64)
    w_n3_, b_n3_ = args["w_n3"].astype(f64), args["b_n3"].astype(f64)
    w_l1_, b_l1_ = args["w_l1"].astype(f64), args["b_l1"].astype(f64)

    b1_eff = b_in1_ + i_flat[0] * w_in1_[H_DIM + S_DIM]
    b_u = b_e1_ + b_in2_ @ w_e1_[:MID]
    b_v = b_in2_ @ w_e1_[MID:]
    b_n1_eff = b_n1_ + b_in2_ @ w_n1_[:MID] + b_e3_ @ w_n1_[MID:]
    b_l1_eff = b_l1_ + b_n3_ @ w_l1_

    wn1b_f = (w_e3_ / (A - 1)) @ w_n1_[MID:]
    wslots = np.zeros((NSLOTS, 128, 128), np.float16)
    wslots[W1T] = w_in1_[:128].astype(np.float16)
    wslots[WE1T] = (w_in2_ @ w_e1_[:MID]).astype(np.float16)
    wslots[WE1B] = (w_in2_ @ w_e1_[MID:]).astype(np.float16)
    wslots[WE2] = args["w_e2"].astype(np.float16)
    wslots[WN1A] = (w_in2_ @ w_n1_[:MID]).astype(np.float16)
    wslots[WN1B] = wn1b_f.astype(np.float16)
    wslots[WN1BN] = (-wn1b_f).astype(np.float16)
    wslots[WN2] = args["w_n2"].astype(np.float16)
    wslots[WL1] = (w_n3_ @ w_l1_).astype(np.float16)
    wslots[WL2, :, :ACT] = args["w_l2"].astype(np.float16)
    w_pack = np.ascontiguousarray(
        wslots.transpose(1, 0, 2).reshape(128, NSLOTS * 128))

    b_pack = np.zeros((128, 8), np.float32)
    for idx, vec in ((B1, b1_eff), (BU, b_u), (BV, b_v),
                     (BE2, args["b_e2"]), (BN1, b_n1_eff),
                     (BN2, args["b_n2"]), (BL1, b_l1_eff)):
        b_pack[:, idx] = np.asarray(vec, np.float32)
    b_pack[:ACT, BL2] = args["b_l2"].astype(np.float32)

    # node features, feat-major, (a, p) column order, per-core shards
    n_all = B * P * A
    X = np.concatenate([args["theta"].reshape(n_all, H_DIM),
                        args["s"].reshape(n_all, S_DIM)], axis=-1)
    in_maps = []
    for c in range(N_CORES):
        xc = X[c * NODES:(c + 1) * NODES]
        in_maps.append({
            "x_fm": _to_ap_major(xc).astype(np.float16),
            "w_pack": w_pack,
            "b_pack": b_pack,
        })

    nc = _get_program()
    if os.environ.get("KERNEL_SIM", "0") == "1":
        # CoreSim core 0 only (cores are identical up to data); other cores
        # return zeros. For correctness devloop, not grading.
        from concourse import bass_interp
        sim = bass_interp.CoreSim(nc)
        for k, v in in_maps[0].items():
            sim.tensor(k)[:] = v
        sim.simulate()
        results = [{"out": np.array(sim.tensor("out"))}]
        results += [{"out": np.zeros((ACT, NODES), np.float32)}
                    for _ in range(N_CORES - 1)]
        parts = [_from_ap_major(r["out"]) for r in results]
        return np.concatenate(parts, axis=0).reshape(B, P, A, ACT).astype(np.float32)

    from concourse.bass_utils import run_bass_kernel_spmd
    trace = os.environ.get("KERNEL_TRACE", "0") == "1"
    res = run_bass_kernel_spmd(nc, in_maps, core_ids=list(range(N_CORES)),
                               trace=trace)
    LAST_EXEC_NS = res.exec_time_ns

    parts = [_from_ap_major(res.results[c]["out"]) for c in range(N_CORES)]
    return np.concatenate(parts, axis=0).reshape(B, P, A, ACT).astype(np.float32)
